# revision 10
# baseline (speedup 1.0000x reference)
"""Trainium2 Bass kernel for CacheShrink MLA attention (8-core SPMD).

Math (matching the reference; dead K/V decompression skipped):
  q = rope(hidden @ Wq) * 1/sqrt(dh)
  c_k, c_v = split(hidden @ Wc)
  per head h (32 heads, GQA onto 4 compressed kv heads):
    S = q_h @ c_k[kv(h)].T  (causal)
    P = exp(S)              (scores are bounded ~[-10, 11], no max needed)
    ctx_h = (P @ c_v[kv(h)]) / rowsum(P)
  out = ctx @ Wo
Sharding: tensor-parallel over heads (4 query heads / 1 compressed-kv
slice per core); bf16 ctx^T shards are AllGather'd and each core
computes a disjoint 512-column block of the output projection.

v2 structure (per-phase, all engines pipelined):
- Phase AB: projection matmuls accumulate in 6 PSUM banks; drains are
  cheap: DVE rope reads q straight out of PSUM via stacked cos/sin
  tables ([cos;sin], [sin;cos] -> 2 muls + cross-half sub/add), ACT
  drains ck/cv, PE transposes cv tiles. PE block-boundary stalls ~1us.
- Phase C: per (t-block, head) unit; QK tiles of unit u are emitted
  interleaved with PV/den tiles of unit u-1 at tile granularity so the
  ACT exp stream (the throughput limiter, ~430ns/tile) never blocks PE
  head-of-line. Diagonal QK matmuls are narrowed to the unmasked
  column range; the causal NEG mask is seeded into the PSUM bank by a
  PE transpose before the QK matmul accumulates on top. den (rowsum
  via ones-vector matmul) and the reciprocal-broadcast rank-1 matmul
  share one PSUM bank.
- AllGather (2 MB/rank bf16) then Phase E: output projection block,
  one PSUM bank per 128-row output tile, ACT drains, DVE idle.

All matmuls bf16 with f32 PSUM accumulation.
"""

import numpy as np
import ml_dtypes

import concourse.bass as bass
import concourse.mybir as mybir
import concourse.tile as tile
from concourse import bacc
from concourse.bass_utils import run_bass_kernel_spmd

BF16 = mybir.dt.bfloat16
F32 = mybir.dt.float32
PSUM = bass.MemorySpace.PSUM

N_CORES = 8
H_PER_CORE = 4      # query heads per core
DH = 128            # head dim
NKO = 32            # k-tiles over the 4096 ctx rows of Wo (32 heads * 128)
TB = 512            # t-block width (one PSUM bank of f32)
NEG = -1.0e30


def build_nc(T=2048, DM=4096, repeat=1, collective=True, hw_loop=True):
    """Build the single-core SPMD program (same for all 8 cores).

    repeat: int (all phases) or (r_ab, r_c, r_ag, r_e) for timing bisects.
    With hw_loop=True, phase repeats >1 run under a tc.For_i hardware loop
    (constant program size); the collective repeat is always unrolled
    (collectives deadlock inside control flow on this runtime).
    """
    if isinstance(repeat, int):
        r_ab = r_c = r_ag = r_e = repeat
    else:
        r_ab, r_c, r_ag, r_e = repeat
    NB = T // TB          # 512-wide t blocks
    JB = TB // 128        # 128-wide s tiles per t block
    NK = DM // 128        # k-tiles over d_model
    NK2 = NK // 2
    NKO2 = NKO // 2
    Exp = mybir.ActivationFunctionType.Exp

    nc = bacc.Bacc("TRN2", target_bir_lowering=True, debug=False,
                   num_devices=N_CORES)

    hT = nc.dram_tensor("hT", [DM, T], BF16, kind="ExternalInput")
    wq = nc.dram_tensor("wq", [DM, H_PER_CORE * DH], BF16,
                        kind="ExternalInput")
    wck = nc.dram_tensor("wck", [DM, DH], BF16, kind="ExternalInput")
    wcv = nc.dram_tensor("wcv", [DM, DH], BF16, kind="ExternalInput")
    wo = nc.dram_tensor("wo", [NKO * 128, H_PER_CORE * DH], BF16,
                        kind="ExternalInput")
    csT = nc.dram_tensor("csT", [128, T], F32, kind="ExternalInput")
    scT = nc.dram_tensor("scT", [128, T], F32, kind="ExternalInput")
    o_t = nc.dram_tensor("o_t", [H_PER_CORE * DH, T], F32,
                         kind="ExternalOutput")

    # internal DRAM for the ctx all-gather
    ctx_loc = nc.dram_tensor("ctx_loc", [H_PER_CORE * DH, T], BF16)
    ctx_all = nc.dram_tensor("ctx_all", [NKO * 128, T], BF16,
                             addr_space="Shared" if collective else "Local")

    hTr = hT.rearrange("(k p) t -> p k t", p=128)
    ctx_all_r = ctx_all.rearrange("(k p) t -> p k t", p=128)
    o_t_r = o_t.rearrange("(m p) t -> p m t", p=128)

    with tile.TileContext(nc) as tc:
        with tc.tile_pool(name="persist", bufs=1) as pp:
            # constants
            identf = pp.tile([128, 128], F32, tag="identf")
            nc.gpsimd.memset(identf[:], 0.0)
            nc.gpsimd.affine_select(
                out=identf[:], in_=identf[:],
                compare_op=mybir.AluOpType.not_equal, fill=1.0,
                base=0, pattern=[[-1, 128]], channel_multiplier=1)
            identb = pp.tile([128, 128], BF16, tag="identb")
            nc.gpsimd.memset(identb[:], 0.0)
            nc.gpsimd.affine_select(
                out=identb[:], in_=identb[:],
                compare_op=mybir.AluOpType.not_equal, fill=1.0,
                base=0, pattern=[[-1, 128]], channel_multiplier=1)
            # maskaddT[t, s] = NEG if s > t else 0; its PE transpose seeds
            # diagonal ST banks with mask[s, t] = NEG above the diagonal.
            maskaddT = pp.tile([128, 128], F32, tag="maskaddT")
            nc.gpsimd.memset(maskaddT[:], 0.0)
            nc.gpsimd.affine_select(
                out=maskaddT[:], in_=maskaddT[:],
                compare_op=mybir.AluOpType.is_ge, fill=NEG,
                base=0, pattern=[[-1, 128]], channel_multiplier=1)
            ones = pp.tile([128, 1], BF16, tag="ones")
            nc.gpsimd.memset(ones[:], 1.0)
            onesrow = pp.tile([1, 128], F32, tag="onesrow")
            nc.gpsimd.memset(onesrow[:], 1.0)

            # weights + stacked rope tables (resident)
            wq_sb = pp.tile([128, NK, H_PER_CORE * DH], BF16, tag="wq")
            nc.sync.dma_start(wq_sb[:], wq.rearrange("(k p) n -> p k n", p=128))
            wck_sb = pp.tile([128, NK, DH], BF16, tag="wck")
            nc.sync.dma_start(wck_sb[:], wck.rearrange("(k p) n -> p k n", p=128))
            wcv_sb = pp.tile([128, NK, DH], BF16, tag="wcv")
            nc.sync.dma_start(wcv_sb[:], wcv.rearrange("(k p) n -> p k n", p=128))
            wo_sb = pp.tile([128, NKO, H_PER_CORE * DH], BF16, tag="wo")
            nc.sync.dma_start(wo_sb[:], wo.rearrange("(k p) n -> p k n", p=128))
            cs_sb = pp.tile([128, T], F32, tag="cs")
            nc.sync.dma_start(cs_sb[:], csT[:])
            sc_sb = pp.tile([128, T], F32, tag="sc")
            nc.sync.dma_start(sc_sb[:], scT[:])

            # per-core activations (persist across phases)
            qrT = [pp.tile([128, T], BF16, tag=f"qrT{h}", name=f"qrT{h}")
                   for h in range(H_PER_CORE)]
            ckT_sb = pp.tile([128, T], BF16, tag="ckT")
            cv_sb = pp.tile([128, T], BF16, tag="cv")  # T/128 tiles [s128, d128]

            def rep(r, body):
                if r == 1:
                    body()
                elif hw_loop:
                    with tc.For_i(0, r, 1):
                        body()
                else:
                    for _ in range(r):
                        body()

            # ---- Phase AB: q/ck/cv projections (+rope) ----
            with (
                tc.tile_pool(name="slab", bufs=3) as slabp,
                tc.tile_pool(name="abw", bufs=4) as abw,
                tc.tile_pool(name="qps", bufs=4, space=PSUM) as qpsp,
                tc.tile_pool(name="kvps", bufs=2, space=PSUM) as kvpsp,
                tc.tile_pool(name="trps", bufs=1, space=PSUM) as trpsp,
            ):
                def ab_body():
                    for b in range(NB):
                        bc = slice(b * TB, (b + 1) * TB)
                        qps = [qpsp.tile([128, TB], F32, tag="q",
                                         name=f"qps{b}_{hh}")
                               for hh in range(H_PER_CORE)]
                        ckp = kvpsp.tile([128, TB], F32, tag="ckv")
                        cvp = kvpsp.tile([128, TB], F32, tag="ckv")
                        for g in range(2):
                            slab = slabp.tile([128, NK2, TB], BF16, tag="slab")
                            nc.sync.dma_start(
                                slab[:], hTr[:, g * NK2:(g + 1) * NK2, bc])
                            for k2 in range(NK2):
                                k = g * NK2 + k2
                                st = (k == 0)
                                sp = (k == NK - 1)
                                for h in range(H_PER_CORE):
                                    nc.tensor.matmul(
                                        qps[h][:],
                                        wq_sb[:, k, h * DH:(h + 1) * DH],
                                        slab[:, k2, :], start=st, stop=sp)
                                nc.tensor.matmul(ckp[:], wck_sb[:, k, :],
                                                 slab[:, k2, :], start=st, stop=sp)
                                nc.tensor.matmul(cvp[:], wcv_sb[:, k, :],
                                                 slab[:, k2, :], start=st, stop=sp)
                        # rope drain straight out of PSUM (cos = cs[0:64],
                        # sin = sc[0:64], both at base partition 0; 1/sqrt(dh)
                        # folded into the tables)
                        for h in range(H_PER_CORE):
                            t1 = abw.tile([64, TB], F32, tag="t1")
                            t2 = abw.tile([64, TB], F32, tag="t2")
                            nc.vector.tensor_mul(t1[:], qps[h][0:64, :],
                                                 cs_sb[0:64, bc])
                            nc.vector.tensor_mul(t2[:], qps[h][64:128, :],
                                                 sc_sb[0:64, bc])
                            nc.vector.tensor_sub(qrT[h][0:64, bc], t1[:], t2[:])
                            t3 = abw.tile([64, TB], F32, tag="t1")
                            t4 = abw.tile([64, TB], F32, tag="t2")
                            nc.vector.tensor_mul(t3[:], qps[h][64:128, :],
                                                 cs_sb[0:64, bc])
                            nc.vector.tensor_mul(t4[:], qps[h][0:64, :],
                                                 sc_sb[0:64, bc])
                            nc.vector.tensor_add(qrT[h][64:128, bc],
                                                 t3[:], t4[:])
                        nc.scalar.copy(ckT_sb[:, bc], ckp[:])
                        cvt = abw.tile([128, TB], BF16, tag="cvt")
                        nc.scalar.copy(cvt[:], cvp[:])
                        trp = trpsp.tile([128, JB, 128], BF16, tag="tr")
                        for jl in range(JB):
                            nc.tensor.transpose(
                                trp[:, jl, :], cvt[:, jl * 128:(jl + 1) * 128],
                                identb[:])
                        nc.vector.tensor_copy(cv_sb[:, bc], trp[:])

                rep(r_ab, ab_body)

            # ---- Phase C: attention (transposed layout) ----
            # Per (head, t-block) unit: pass 1 computes all QK tiles and
            # exps them into SBUF probs; pass 2 runs the PV/denominator
            # accumulation. Units are software-pipelined one deep so the
            # ACT exps of unit u+1 overlap the PE PV pass of unit u.
            with (
                tc.tile_pool(name="cwork", bufs=3) as cw,
                tc.tile_pool(name="probs", bufs=36) as prp,
                tc.tile_pool(name="stps", bufs=3, space=PSUM) as stp,
                tc.tile_pool(name="ctxps", bufs=2, space=PSUM) as ctxp,
                tc.tile_pool(name="denps", bufs=2, space=PSUM) as denp,
                tc.tile_pool(name="bcps", bufs=1, space=PSUM) as bcp,
            ):
                units = [(h, b) for h in range(H_PER_CORE)
                         for b in range(NB)]

                def c_body():
                    def qk_pass(h, b):
                        nj = JB * (b + 1)
                        probs = []
                        for j in range(nj):
                            c = j - JB * b          # >=0 on diagonal tiles
                            lo = 128 * max(c, 0)
                            stps = stp.tile([128, TB], F32, tag="st",
                                            name=f"st{h}_{b}_{j}")
                            if c >= 0:
                                # seed bank: clears has_written, writes NEG
                                # mask above diagonal in cols [lo:lo+128]
                                nc.tensor.transpose(
                                    stps[:, lo:lo + 128], maskaddT[:],
                                    identf[:])
                                nc.tensor.matmul(
                                    stps[:],
                                    ckT_sb[:, j * 128:(j + 1) * 128],
                                    qrT[h][:, b * TB:(b + 1) * TB],
                                    start=False, stop=True,
                                    skip_group_check=True)
                            else:
                                nc.tensor.matmul(
                                    stps[:],
                                    ckT_sb[:, j * 128:(j + 1) * 128],
                                    qrT[h][:, b * TB:(b + 1) * TB],
                                    start=True, stop=True)
                            pr = prp.tile([128, TB], BF16, tag="probs",
                                          name=f"pr{h}_{b}_{j}")
                            nc.scalar.activation(pr[:, lo:], stps[:, lo:], Exp)
                            probs.append((j, lo, pr))
                        return probs

                    def pv_pass(h, b, probs):
                        nj = JB * (b + 1)
                        ctxps = ctxp.tile([128, TB], F32, tag="ctx",
                                          name=f"ctx{h}_{b}")
                        denps = denp.tile([1, TB], F32, tag="den",
                                          name=f"den{h}_{b}")
                        for (j, lo, pr) in probs:
                            nc.tensor.matmul(
                                ctxps[:, lo:], cv_sb[:, j * 128:(j + 1) * 128],
                                pr[:, lo:],
                                start=(j == 0), stop=(j == nj - 1))
                            nc.tensor.matmul(
                                denps[:, lo:], ones[:], pr[:, lo:],
                                start=(j == 0), stop=(j == nj - 1))
                        rec = cw.tile([1, TB], F32, tag="rec")
                        nc.vector.reciprocal(rec[:], denps[:])
                        bc_ps = bcp.tile([128, TB], F32, tag="bc")
                        nc.tensor.matmul(bc_ps[:], onesrow[:], rec[:])
                        bcs = cw.tile([128, TB], F32, tag="bcs")
                        nc.vector.tensor_copy(bcs[:], bc_ps[:])
                        cn = cw.tile([128, TB], BF16, tag="cn")
                        nc.vector.tensor_mul(cn[:], ctxps[:], bcs[:])
                        nc.sync.dma_start(
                            ctx_loc[h * 128:(h + 1) * 128,
                                    b * TB:(b + 1) * TB], cn[:])

                    prev = None
                    for (h, b) in units:
                        probs = qk_pass(h, b)
                        if prev is not None:
                            pv_pass(*prev)
                        prev = (h, b, probs)
                    pv_pass(*prev)

                rep(r_c, c_body)

            for _ in range(r_ag):
                # ---- AllGather ctx across the 8 cores ----
                if collective:
                    nc.gpsimd.collective_compute(
                        "AllGather", mybir.AluOpType.bypass,
                        ins=[ctx_loc[:]], outs=[ctx_all[:]],
                        replica_groups=[list(range(N_CORES))])

            # ---- Phase E: output projection (512-col block) ----
            with (
                tc.tile_pool(name="cslab", bufs=3) as csp,
                tc.tile_pool(name="ost", bufs=3) as ostp,
                tc.tile_pool(name="ops", bufs=2, space=PSUM) as opsp,
            ):
                def e_body():
                    for b in range(NB):
                        bc = slice(b * TB, (b + 1) * TB)
                        slabs = []
                        for g in range(2):
                            cslab = csp.tile([128, NKO2, TB], BF16, tag="cs")
                            nc.sync.dma_start(
                                cslab[:],
                                ctx_all_r[:, g * NKO2:(g + 1) * NKO2, bc])
                            slabs.append(cslab)
                        for m in range(H_PER_CORE):
                            oacc = opsp.tile([128, TB], F32, tag="o",
                                             name=f"o{b}_{m}")
                            for g in range(2):
                                for k2 in range(NKO2):
                                    k = g * NKO2 + k2
                                    nc.tensor.matmul(
                                        oacc[:],
                                        wo_sb[:, k, m * 128:(m + 1) * 128],
                                        slabs[g][:, k2, :],
                                        start=(k == 0), stop=(k == NKO - 1))
                            ost = ostp.tile([128, TB], F32, tag="ost")
                            nc.scalar.copy(ost[:], oacc[:])
                            nc.sync.dma_start(o_t_r[:, m, bc], ost[:])

                rep(r_e, e_body)

    nc.compile()
    return nc


_CACHE = {}


def _get_nc(T, DM, repeat=1):
    key = (T, DM, repeat)
    if key not in _CACHE:
        _CACHE[key] = build_nc(T, DM, repeat)
    return _CACHE[key]


def make_inputs(positions, hidden_states, Wq, Wc, Wo, T, DM):
    """Shard + prep the full inputs into 8 per-core input maps."""
    bf = ml_dtypes.bfloat16
    d_latent = Wc.shape[1] // 2
    hT = np.ascontiguousarray(hidden_states.T).astype(bf)

    pos = positions.astype(np.float32)
    inv = (1.0 / (10000.0 ** (np.arange(64, dtype=np.float32) * (2.0 / 128.0))))
    freqs = pos[:, None] * inv[None, :]          # (T, 64) f32
    scale = np.float32(1.0 / np.sqrt(128.0))
    cosT = np.ascontiguousarray((np.cos(freqs) * scale).T)  # (64, T)
    sinT = np.ascontiguousarray((np.sin(freqs) * scale).T)
    csT = np.concatenate([cosT, sinT], axis=0)   # (128, T): [cos; sin]
    scT = np.concatenate([sinT, cosT], axis=0)   # (128, T): [sin; cos]

    in_maps = []
    for i in range(N_CORES):
        kv = i // 2
        in_maps.append({
            "hT": hT,
            "wq": np.ascontiguousarray(
                Wq[:, i * H_PER_CORE * DH:(i + 1) * H_PER_CORE * DH]).astype(bf),
            "wck": np.ascontiguousarray(
                Wc[:, kv * DH:(kv + 1) * DH]).astype(bf),
            "wcv": np.ascontiguousarray(
                Wc[:, d_latent + kv * DH:d_latent + (kv + 1) * DH]).astype(bf),
            "wo": np.ascontiguousarray(
                Wo[:, i * H_PER_CORE * DH:(i + 1) * H_PER_CORE * DH]).astype(bf),
            "csT": csT,
            "scT": scT,
        })
    return in_maps


def kernel(positions, hidden_states, Wq, Wc, Wuk, Wuv, Wo):
    positions = np.asarray(positions)
    hidden_states = np.asarray(hidden_states, dtype=np.float32)
    Wq = np.asarray(Wq, dtype=np.float32)
    Wc = np.asarray(Wc, dtype=np.float32)
    Wo = np.asarray(Wo, dtype=np.float32)
    T, DM = hidden_states.shape

    nc = _get_nc(T, DM)
    in_maps = make_inputs(positions, hidden_states, Wq, Wc, Wo, T, DM)
    res = run_bass_kernel_spmd(nc, in_maps, list(range(N_CORES))).results
    oT = np.concatenate([res[i]["o_t"] for i in range(N_CORES)], axis=0)
    return np.ascontiguousarray(oT.T)


# revision 11
# speedup vs baseline: 2.1080x; 2.1080x over previous
"""Trainium2 Bass kernel for CacheShrink MLA attention (8-core SPMD).

Math (matching the reference; dead K/V decompression skipped):
  q = rope(hidden @ Wq) * 1/sqrt(dh)
  c_k, c_v = split(hidden @ Wc)
  per head h (32 heads, GQA onto 4 compressed kv heads):
    S = q_h @ c_k[kv(h)].T  (causal)
    P = exp(S)              (scores are bounded ~[-10, 11], no max needed)
    ctx_h = (P @ c_v[kv(h)]) / rowsum(P)
  out = ctx @ Wo
Sharding: tensor-parallel over heads (4 query heads / 1 compressed-kv
slice per core); bf16 ctx^T shards are AllGather'd and each core
computes a disjoint 512-column block of the output projection.

v2 structure (per-phase, all engines pipelined):
- Phase AB: projection matmuls accumulate in 6 PSUM banks; drains are
  cheap: DVE rope reads q straight out of PSUM via stacked cos/sin
  tables ([cos;sin], [sin;cos] -> 2 muls + cross-half sub/add), ACT
  drains ck/cv, PE transposes cv tiles. PE block-boundary stalls ~1us.
- Phase C: per (t-block, head) unit; QK tiles of unit u are emitted
  interleaved with PV/den tiles of unit u-1 at tile granularity so the
  ACT exp stream (the throughput limiter, ~430ns/tile) never blocks PE
  head-of-line. Diagonal QK matmuls are narrowed to the unmasked
  column range; the causal NEG mask is seeded into the PSUM bank by a
  PE transpose before the QK matmul accumulates on top. den (rowsum
  via ones-vector matmul) and the reciprocal-broadcast rank-1 matmul
  share one PSUM bank.
- AllGather (2 MB/rank bf16) then Phase E: output projection block,
  one PSUM bank per 128-row output tile, ACT drains, DVE idle.

All matmuls bf16 with f32 PSUM accumulation.
"""

import numpy as np
import ml_dtypes

import concourse.bass as bass
import concourse.mybir as mybir
import concourse.tile as tile
from concourse import bacc
from concourse.bass_utils import run_bass_kernel_spmd

BF16 = mybir.dt.bfloat16
F32 = mybir.dt.float32
PSUM = bass.MemorySpace.PSUM

N_CORES = 8
H_PER_CORE = 4      # query heads per core
DH = 128            # head dim
NKO = 32            # k-tiles over the 4096 ctx rows of Wo (32 heads * 128)
TB = 512            # t-block width (one PSUM bank of f32)
NEG = -1.0e30


def build_nc(T=2048, DM=4096, repeat=1, collective=True, hw_loop=True):
    """Build the single-core SPMD program (same for all 8 cores).

    repeat: int (all phases) or (r_ab, r_c, r_ag, r_e) for timing bisects.
    With hw_loop=True, phase repeats >1 run under a tc.For_i hardware loop
    (constant program size); the collective repeat is always unrolled
    (collectives deadlock inside control flow on this runtime).
    """
    if isinstance(repeat, int):
        r_ab = r_c = r_ag = r_e = repeat
    else:
        r_ab, r_c, r_ag, r_e = repeat
    NB = T // TB          # 512-wide t blocks
    JB = TB // 128        # 128-wide s tiles per t block
    NK = DM // 128        # k-tiles over d_model
    NK2 = NK // 2
    NKO2 = NKO // 2
    Exp = mybir.ActivationFunctionType.Exp

    nc = bacc.Bacc("TRN2", target_bir_lowering=True, debug=False,
                   num_devices=N_CORES)

    hT = nc.dram_tensor("hT", [DM, T], BF16, kind="ExternalInput")
    wq = nc.dram_tensor("wq", [DM, H_PER_CORE * DH], BF16,
                        kind="ExternalInput")
    wck = nc.dram_tensor("wck", [DM, DH], BF16, kind="ExternalInput")
    wcv = nc.dram_tensor("wcv", [DM, DH], BF16, kind="ExternalInput")
    wo = nc.dram_tensor("wo", [NKO * 128, H_PER_CORE * DH], BF16,
                        kind="ExternalInput")
    csT = nc.dram_tensor("csT", [128, T], F32, kind="ExternalInput")
    scT = nc.dram_tensor("scT", [128, T], F32, kind="ExternalInput")
    o_t = nc.dram_tensor("o_t", [H_PER_CORE * DH, T], F32,
                         kind="ExternalOutput")

    # internal DRAM for the ctx all-gather
    ctx_loc = nc.dram_tensor("ctx_loc", [H_PER_CORE * DH, T], BF16)
    ctx_all = nc.dram_tensor("ctx_all", [NKO * 128, T], BF16,
                             addr_space="Shared" if collective else "Local")

    hTr = hT.rearrange("(k p) t -> p k t", p=128)
    ctx_all_r = ctx_all.rearrange("(k p) t -> p k t", p=128)
    o_t_r = o_t.rearrange("(m p) t -> p m t", p=128)

    with tile.TileContext(nc) as tc:
        with tc.tile_pool(name="persist", bufs=1) as pp:
            # constants
            identf = pp.tile([128, 128], F32, tag="identf")
            nc.gpsimd.memset(identf[:], 0.0)
            nc.gpsimd.affine_select(
                out=identf[:], in_=identf[:],
                compare_op=mybir.AluOpType.not_equal, fill=1.0,
                base=0, pattern=[[-1, 128]], channel_multiplier=1)
            identb = pp.tile([128, 128], BF16, tag="identb")
            nc.gpsimd.memset(identb[:], 0.0)
            nc.gpsimd.affine_select(
                out=identb[:], in_=identb[:],
                compare_op=mybir.AluOpType.not_equal, fill=1.0,
                base=0, pattern=[[-1, 128]], channel_multiplier=1)
            # maskaddT[t, s] = NEG if s > t else 0; its PE transpose seeds
            # diagonal ST banks with mask[s, t] = NEG above the diagonal.
            maskaddT = pp.tile([128, 128], F32, tag="maskaddT")
            nc.gpsimd.memset(maskaddT[:], 0.0)
            nc.gpsimd.affine_select(
                out=maskaddT[:], in_=maskaddT[:],
                compare_op=mybir.AluOpType.is_ge, fill=NEG,
                base=0, pattern=[[-1, 128]], channel_multiplier=1)
            ones = pp.tile([128, 1], BF16, tag="ones")
            nc.gpsimd.memset(ones[:], 1.0)
            onesrow = pp.tile([1, 128], F32, tag="onesrow")
            nc.gpsimd.memset(onesrow[:], 1.0)

            # weights + stacked rope tables (resident)
            wq_sb = pp.tile([128, NK, H_PER_CORE * DH], BF16, tag="wq")
            nc.sync.dma_start(wq_sb[:], wq.rearrange("(k p) n -> p k n", p=128))
            wck_sb = pp.tile([128, NK, DH], BF16, tag="wck")
            nc.sync.dma_start(wck_sb[:], wck.rearrange("(k p) n -> p k n", p=128))
            wcv_sb = pp.tile([128, NK, DH], BF16, tag="wcv")
            nc.sync.dma_start(wcv_sb[:], wcv.rearrange("(k p) n -> p k n", p=128))
            wo_sb = pp.tile([128, NKO, H_PER_CORE * DH], BF16, tag="wo")
            nc.sync.dma_start(wo_sb[:], wo.rearrange("(k p) n -> p k n", p=128))
            cs_sb = pp.tile([128, T], F32, tag="cs")
            nc.sync.dma_start(cs_sb[:], csT[:])
            sc_sb = pp.tile([128, T], F32, tag="sc")
            nc.sync.dma_start(sc_sb[:], scT[:])

            # per-core activations (persist across phases)
            qrT = [pp.tile([128, T], BF16, tag=f"qrT{h}", name=f"qrT{h}")
                   for h in range(H_PER_CORE)]
            ckT_sb = pp.tile([128, T], BF16, tag="ckT")
            cv_sb = pp.tile([128, T], BF16, tag="cv")  # T/128 tiles [s128, d128]

            def rep(r, body):
                if r == 1:
                    body()
                elif hw_loop:
                    with tc.For_i(0, r, 1):
                        body()
                else:
                    for _ in range(r):
                        body()

            # ---- Phase AB: q/ck/cv projections (+rope) ----
            with (
                tc.tile_pool(name="slab", bufs=3) as slabp,
                tc.tile_pool(name="abw", bufs=4) as abw,
                tc.tile_pool(name="qps", bufs=4, space=PSUM) as qpsp,
                tc.tile_pool(name="kvps", bufs=2, space=PSUM) as kvpsp,
                tc.tile_pool(name="trps", bufs=1, space=PSUM) as trpsp,
            ):
                def ab_body():
                    for b in range(NB):
                        bc = slice(b * TB, (b + 1) * TB)
                        qps = [qpsp.tile([128, TB], F32, tag="q",
                                         name=f"qps{b}_{hh}")
                               for hh in range(H_PER_CORE)]
                        ckp = kvpsp.tile([128, TB], F32, tag="ckv")
                        cvp = kvpsp.tile([128, TB], F32, tag="ckv")
                        for g in range(2):
                            slab = slabp.tile([128, NK2, TB], BF16, tag="slab")
                            nc.sync.dma_start(
                                slab[:], hTr[:, g * NK2:(g + 1) * NK2, bc])
                            for k2 in range(NK2):
                                k = g * NK2 + k2
                                st = (k == 0)
                                sp = (k == NK - 1)
                                for h in range(H_PER_CORE):
                                    nc.tensor.matmul(
                                        qps[h][:],
                                        wq_sb[:, k, h * DH:(h + 1) * DH],
                                        slab[:, k2, :], start=st, stop=sp)
                                nc.tensor.matmul(ckp[:], wck_sb[:, k, :],
                                                 slab[:, k2, :], start=st, stop=sp)
                                nc.tensor.matmul(cvp[:], wcv_sb[:, k, :],
                                                 slab[:, k2, :], start=st, stop=sp)
                        # q drain: ACT copies PSUM->SBUF fast (~0.4us/tile)
                        # so the PE can reuse the bank immediately; DVE ropes
                        # from SBUF at leisure, overlapped with the next
                        # block's matmuls. The stacked tables make every DVE
                        # op base-partition-aligned: cs = [cos;sin],
                        # sc = [sin;cos] (1/sqrt(dh) folded in), so
                        #   q1*cos = qsb[0:64]*cs[0:64]
                        #   q2*sin = qsb[64:]*cs[64:]
                        #   q2*cos = qsb[64:]*sc[64:]
                        #   q1*sin = qsb[0:64]*sc[0:64]
                        for h in range(H_PER_CORE):
                            qsb = abw.tile([128, TB], F32, tag="qsb")
                            nc.scalar.copy(qsb[:], qps[h][:])
                            t1 = abw.tile([64, TB], F32, tag="t1")
                            t2 = abw.tile([64, TB], F32, tag="t2")
                            nc.vector.tensor_mul(t1[:], qsb[0:64, :],
                                                 cs_sb[0:64, bc])
                            nc.vector.tensor_mul(t2[:], qsb[64:128, :],
                                                 cs_sb[64:128, bc])
                            nc.vector.tensor_sub(qrT[h][0:64, bc], t1[:], t2[:])
                            t3 = abw.tile([64, TB], F32, tag="t1")
                            t4 = abw.tile([64, TB], F32, tag="t2")
                            nc.vector.tensor_mul(t3[:], qsb[64:128, :],
                                                 sc_sb[64:128, bc])
                            nc.vector.tensor_mul(t4[:], qsb[0:64, :],
                                                 sc_sb[0:64, bc])
                            nc.vector.tensor_add(qrT[h][64:128, bc],
                                                 t3[:], t4[:])
                        nc.scalar.copy(ckT_sb[:, bc], ckp[:])
                        cvt = abw.tile([128, TB], BF16, tag="cvt")
                        nc.scalar.copy(cvt[:], cvp[:])
                        trp = trpsp.tile([128, JB, 128], BF16, tag="tr")
                        for jl in range(JB):
                            nc.tensor.transpose(
                                trp[:, jl, :], cvt[:, jl * 128:(jl + 1) * 128],
                                identb[:])
                        nc.vector.tensor_copy(cv_sb[:, bc], trp[:])

                rep(r_ab, ab_body)

            # ---- Phase C: attention (transposed layout) ----
            # Per (head, t-block) unit: pass 1 computes all QK tiles and
            # exps them into SBUF probs; pass 2 runs the PV/denominator
            # accumulation. Units are software-pipelined one deep so the
            # ACT exps of unit u+1 overlap the PE PV pass of unit u.
            with (
                tc.tile_pool(name="cwork", bufs=3) as cw,
                tc.tile_pool(name="probs", bufs=36) as prp,
                tc.tile_pool(name="stps", bufs=3, space=PSUM) as stp,
                tc.tile_pool(name="ctxps", bufs=2, space=PSUM) as ctxp,
                tc.tile_pool(name="denps", bufs=2, space=PSUM) as denp,
                tc.tile_pool(name="bcps", bufs=1, space=PSUM) as bcp,
            ):
                units = [(h, b) for h in range(H_PER_CORE)
                         for b in range(NB)]

                def c_body():
                    def qk_pass(h, b):
                        nj = JB * (b + 1)
                        probs = []
                        for j in range(nj):
                            c = j - JB * b          # >=0 on diagonal tiles
                            lo = 128 * max(c, 0)
                            stps = stp.tile([128, TB], F32, tag="st",
                                            name=f"st{h}_{b}_{j}")
                            if c >= 0:
                                # seed bank: clears has_written, writes NEG
                                # mask above diagonal in cols [lo:lo+128]
                                nc.tensor.transpose(
                                    stps[:, lo:lo + 128], maskaddT[:],
                                    identf[:])
                                nc.tensor.matmul(
                                    stps[:],
                                    ckT_sb[:, j * 128:(j + 1) * 128],
                                    qrT[h][:, b * TB:(b + 1) * TB],
                                    start=False, stop=True,
                                    skip_group_check=True)
                            else:
                                nc.tensor.matmul(
                                    stps[:],
                                    ckT_sb[:, j * 128:(j + 1) * 128],
                                    qrT[h][:, b * TB:(b + 1) * TB],
                                    start=True, stop=True)
                            pr = prp.tile([128, TB], BF16, tag="probs",
                                          name=f"pr{h}_{b}_{j}")
                            nc.scalar.activation(pr[:, lo:], stps[:, lo:], Exp)
                            probs.append((j, lo, pr))
                        return probs

                    def pv_pass(h, b, probs):
                        nj = JB * (b + 1)
                        ctxps = ctxp.tile([128, TB], F32, tag="ctx",
                                          name=f"ctx{h}_{b}")
                        denps = denp.tile([1, TB], F32, tag="den",
                                          name=f"den{h}_{b}")
                        for (j, lo, pr) in probs:
                            nc.tensor.matmul(
                                ctxps[:, lo:], cv_sb[:, j * 128:(j + 1) * 128],
                                pr[:, lo:],
                                start=(j == 0), stop=(j == nj - 1))
                            nc.tensor.matmul(
                                denps[:, lo:], ones[:], pr[:, lo:],
                                start=(j == 0), stop=(j == nj - 1))
                        rec = cw.tile([1, TB], F32, tag="rec")
                        nc.vector.reciprocal(rec[:], denps[:])
                        bc_ps = bcp.tile([128, TB], F32, tag="bc")
                        nc.tensor.matmul(bc_ps[:], onesrow[:], rec[:])
                        bcs = cw.tile([128, TB], F32, tag="bcs")
                        nc.vector.tensor_copy(bcs[:], bc_ps[:])
                        cn = cw.tile([128, TB], BF16, tag="cn")
                        nc.vector.tensor_mul(cn[:], ctxps[:], bcs[:])
                        nc.sync.dma_start(
                            ctx_loc[h * 128:(h + 1) * 128,
                                    b * TB:(b + 1) * TB], cn[:])

                    prev = None
                    for (h, b) in units:
                        probs = qk_pass(h, b)
                        if prev is not None:
                            pv_pass(*prev)
                        prev = (h, b, probs)
                    pv_pass(*prev)

                rep(r_c, c_body)

            for _ in range(r_ag):
                # ---- AllGather ctx across the 8 cores ----
                if collective:
                    nc.gpsimd.collective_compute(
                        "AllGather", mybir.AluOpType.bypass,
                        ins=[ctx_loc[:]], outs=[ctx_all[:]],
                        replica_groups=[list(range(N_CORES))])

            # ---- Phase E: output projection (512-col block) ----
            with (
                tc.tile_pool(name="cslab", bufs=3) as csp,
                tc.tile_pool(name="ost", bufs=3) as ostp,
                tc.tile_pool(name="ops", bufs=2, space=PSUM) as opsp,
            ):
                def e_body():
                    for b in range(NB):
                        bc = slice(b * TB, (b + 1) * TB)
                        slabs = []
                        for g in range(2):
                            cslab = csp.tile([128, NKO2, TB], BF16, tag="cs")
                            nc.sync.dma_start(
                                cslab[:],
                                ctx_all_r[:, g * NKO2:(g + 1) * NKO2, bc])
                            slabs.append(cslab)
                        for m in range(H_PER_CORE):
                            oacc = opsp.tile([128, TB], F32, tag="o",
                                             name=f"o{b}_{m}")
                            for g in range(2):
                                for k2 in range(NKO2):
                                    k = g * NKO2 + k2
                                    nc.tensor.matmul(
                                        oacc[:],
                                        wo_sb[:, k, m * 128:(m + 1) * 128],
                                        slabs[g][:, k2, :],
                                        start=(k == 0), stop=(k == NKO - 1))
                            ost = ostp.tile([128, TB], F32, tag="ost")
                            nc.scalar.copy(ost[:], oacc[:])
                            nc.sync.dma_start(o_t_r[:, m, bc], ost[:])

                rep(r_e, e_body)

    nc.compile()
    return nc


_CACHE = {}


def _get_nc(T, DM, repeat=1):
    key = (T, DM, repeat)
    if key not in _CACHE:
        _CACHE[key] = build_nc(T, DM, repeat)
    return _CACHE[key]


def make_inputs(positions, hidden_states, Wq, Wc, Wo, T, DM):
    """Shard + prep the full inputs into 8 per-core input maps."""
    bf = ml_dtypes.bfloat16
    d_latent = Wc.shape[1] // 2
    hT = np.ascontiguousarray(hidden_states.T).astype(bf)

    pos = positions.astype(np.float32)
    inv = (1.0 / (10000.0 ** (np.arange(64, dtype=np.float32) * (2.0 / 128.0))))
    freqs = pos[:, None] * inv[None, :]          # (T, 64) f32
    scale = np.float32(1.0 / np.sqrt(128.0))
    cosT = np.ascontiguousarray((np.cos(freqs) * scale).T)  # (64, T)
    sinT = np.ascontiguousarray((np.sin(freqs) * scale).T)
    csT = np.concatenate([cosT, sinT], axis=0)   # (128, T): [cos; sin]
    scT = np.concatenate([sinT, cosT], axis=0)   # (128, T): [sin; cos]

    in_maps = []
    for i in range(N_CORES):
        kv = i // 2
        in_maps.append({
            "hT": hT,
            "wq": np.ascontiguousarray(
                Wq[:, i * H_PER_CORE * DH:(i + 1) * H_PER_CORE * DH]).astype(bf),
            "wck": np.ascontiguousarray(
                Wc[:, kv * DH:(kv + 1) * DH]).astype(bf),
            "wcv": np.ascontiguousarray(
                Wc[:, d_latent + kv * DH:d_latent + (kv + 1) * DH]).astype(bf),
            "wo": np.ascontiguousarray(
                Wo[:, i * H_PER_CORE * DH:(i + 1) * H_PER_CORE * DH]).astype(bf),
            "csT": csT,
            "scT": scT,
        })
    return in_maps


def kernel(positions, hidden_states, Wq, Wc, Wuk, Wuv, Wo):
    positions = np.asarray(positions)
    hidden_states = np.asarray(hidden_states, dtype=np.float32)
    Wq = np.asarray(Wq, dtype=np.float32)
    Wc = np.asarray(Wc, dtype=np.float32)
    Wo = np.asarray(Wo, dtype=np.float32)
    T, DM = hidden_states.shape

    nc = _get_nc(T, DM)
    in_maps = make_inputs(positions, hidden_states, Wq, Wc, Wo, T, DM)
    res = run_bass_kernel_spmd(nc, in_maps, list(range(N_CORES))).results
    oT = np.concatenate([res[i]["o_t"] for i in range(N_CORES)], axis=0)
    return np.ascontiguousarray(oT.T)
